# revision 41
# baseline (speedup 1.0000x reference)
"""Binary-tree gated-expert MoE kernel for 8 Trainium2 NeuronCores.

Reference computation (B=4096, D=2048, 4 levels, 1/2/4/8 experts):
    h = x
    for level l: h = relu(h @ Wl[eid_l] + bl[eid_l])
where eid_l is the l-bit prefix of the 3-bit leaf id built from
path_mask[:, 0:3].

Strategy: sibling-paired half-leaf dispatch.  Samples are grouped by
leaf; sibling leaves (2j, 2j+1) share their level-0..2 experts and
differ only at level 3.  Each of the two cores serving pair j takes
half of leaf 2j in PSUM column-chunk 0 and half of leaf 2j+1 in chunk
1.  Because the per-(jt,kt) matmul is issued per PSUM chunk anyway
(Bc > 512 needs two chunks), pointing chunk 1 at a second level-3
weight tile costs nothing on the Tensor engine, and both chunks stay
large enough (>= ~240 rows) that the ~97ns LDWEIGHTS pipeline stays
hidden under the matmuls.

Everything flows in bfloat16 (weights, x, inter-level activations and
the output, upconverted on host); PSUM accumulation stays fp32, so the
end-to-end rel-err vs the fp32 reference is ~5e-3, well inside the
2e-2 gate.  bf16 also halves HBM traffic and SBUF footprint.

Streaming layout (one per-core SDMA engine serves all queues and
round-robins packets across every QUEUED TRANSFER, so packet economy,
transfer count and issue order all matter; each engine's instruction
stream is FIFO, so a dependency-gated DMA issue must live on the
engine that produces its dependency):
  - Weights ride the SYNC HW queue.  The first jg streams as 7 fine
    pieces (kt0 | kt1 | kt2-3 | kt4-5 | kt6-7 | half | half) so the
    first matmul starts as early as the ~0.6us/issue cadence allows;
    the first 8 entries carry no dep chain (the issue cadence itself
    paces the queue; a dep chain would add completion->issue round
    trips), later 512KB quarters are chained 4-deep so completion
    order tracks consumption order.
  - x kt0 leads the sync chain (the first matmul needs it to win the
    round-robin packet race); x kt1-7 rides the scalar HW queue
    fire-and-forget in size-ascending pieces, which makes round-robin
    completion order match consumption order; x kt8-15 (most slack)
    joins the sync chain after the first weight group so it never
    steals packet share from the kt4-11 weight pieces.  (Chaining the
    scalar pieces was measured strictly worse; so was one queue.)
  - The bias table is host-linearized to [128, 5*16] so its DMA is one
    320B contiguous run per partition (the naive [5, D] layout shatters
    into 10240 4-byte packets that steal ~45% of the SDMA engine's
    packet slots exactly during the critical first 16us).
  - Level-3 output stores ride the scalar queue (per-jg, last jg
    per-jt): the scalar engine produces the ACTs the stores wait on,
    so the issues never head-of-line block anything.  Stores on the
    sync chain would stall the W3 stream (measured +3.4us x3 + HAM
    re-throttle); SWDGE (GpSimd) is ~50GB/s and delays the teardown.
  - A short warmup matmul burst bridges the HAM clock-gate ramp while
    the first weight pieces land.
"""

import math

import numpy as np
import ml_dtypes

from concourse import bacc, mybir, tile
from concourse.bass_utils import run_bass_kernel_spmd

D = 2048
KT = D // 128          # 16 contraction k-tiles
JT = D // 128          # 16 output-feature blocks
JG = 4                 # j-groups of 4 blocks (512 features) per W DMA
N_CORES = 8
N_LEVELS = 4
F32 = mybir.dt.float32
BF16 = mybir.dt.bfloat16
BF16_NP = ml_dtypes.bfloat16

NQ = 4                 # W DMA split: quarters of 4 k-tiles
PACE_WIN = 4           # max in-flight paced DMAs on the SP ring
WARM_N = 3             # PE p-state warmup matmuls
W_BUFS = 6             # weight tile buffering (deep for L3 dual stream)

_cache: dict = {}


def _build(c0: int, c1: int):
    """Build + compile the per-core Bass program.  PSUM chunk 0 holds
    ``c0`` columns (even-leaf half), chunk 1 ``c1`` columns (odd-leaf
    half); both use one weight stream for levels 0-2 and separate
    level-3 streams W3A / W3B."""
    key = (c0, c1)
    if key in _cache:
        return _cache[key]
    Bc = c0 + c1

    nc = bacc.Bacc("TRN2", target_bir_lowering=False, debug=False,
                   num_devices=N_CORES)

    # Weights arrive host-linearized as [JG, 128, KT*512]:
    # element (jg, p, kt, jc) = W[kt*128 + p, jg*512 + jc], so each DMA
    # reads long contiguous runs per partition.
    # x and out are host-linearized to the exact SBUF per-partition
    # layout [128, KT*Bc]: DMA runs are then len(ks)*Bc*2 bytes
    # contiguous per partition (2-8KB).
    xT = nc.dram_tensor("xT", [128, KT * Bc], BF16, kind="ExternalInput")
    Wshape = [JG, 128, KT * 512]
    Ws = [nc.dram_tensor(f"W{l}", Wshape, BF16, kind="ExternalInput")
          for l in range(N_LEVELS - 1)]
    W3A = nc.dram_tensor("W3A", Wshape, BF16, kind="ExternalInput")
    W3B = nc.dram_tensor("W3B", Wshape, BF16, kind="ExternalInput")
    # bias host-linearized: element (p, l*JT + jt) = b_l[jt*128 + p],
    # one contiguous (N_LEVELS+1)*JT*4B run per partition.
    bias = nc.dram_tensor("bias", [128, (N_LEVELS + 1) * JT], F32,
                          kind="ExternalInput")
    out = nc.dram_tensor("out", [128, JT * Bc], BF16, kind="ExternalOutput")

    xTv = xT.rearrange("p (kt b) -> p kt b", b=Bc)
    outv = out.rearrange("p (jt b) -> p jt b", b=Bc)
    bv = bias.rearrange("p (l jt) -> p l jt", jt=JT)
    KQ = KT // NQ               # k-tiles per quarter
    QW = KQ * 512               # W free-dim elements per quarter

    csl = (slice(0, c0), slice(c0, Bc))

    with tile.TileContext(nc) as tc:
        with (
            tc.tile_pool(name="acts", bufs=1) as acts,
            tc.tile_pool(name="w", bufs=W_BUFS) as wpool,
            tc.tile_pool(name="ps", bufs=8, space="PSUM") as ps,
            tc.tile_pool(name="misc", bufs=1) as misc,
        ):
            actA = acts.tile([128, KT, Bc], BF16, tag="A")
            actB = acts.tile([128, KT, Bc], BF16, tag="B")
            btile = misc.tile([128, N_LEVELS + 1, JT], F32)
            nc.scalar.dma_start(btile[:], bv)

            # Warm the PE HAM clock gate during the DMA lead-in:
            # throwaway matmuls on a zeroed tile so the first real
            # matmul runs at full clock instead of ramping on real
            # work.  Sized to end right when the first real matmul's
            # data lands (~9.4us); the real stream continues the ramp.
            warm = misc.tile([128, 512], BF16)
            nc.gpsimd.memset(warm[:], 0.0)
            wacc = ps.tile([128, 512], F32, tag="ps", name="wacc")
            for i in range(WARM_N):
                nc.tensor.matmul(wacc[:], warm[:, :128], warm[:],
                                 start=(i == 0), stop=(i == WARM_N - 1))

            # Weight DMAs go on the SP ring, chained so a bounded
            # number are in flight.  The HW SDMA engine round-robins
            # packets across every QUEUED TRANSFER (not per queue), so
            # an unbounded backlog makes every transfer finish near the
            # end; a short chain keeps completion order = consumption
            # order once the stream reaches its 512KB steady state.
            paced = []
            NOHEAD = 4   # head entries are small: the ~0.6us issue
            # cadence itself paces the queue, and a dep chain would
            # only add completion->issue round-trip dead time.

            def paced_dma(dst_ap, src_ap):
                h = nc.sync.dma_start(dst_ap, src_ap)
                n = len(paced)
                if n >= NOHEAD + PACE_WIN:
                    tile.add_dep_helper(h.ins, paced[-PACE_WIN].ins,
                                        reason="dma pacing chain")
                paced.append(h)
                return h

            # Input streaming: x kt0 leads the sync chain (the first
            # matmul needs it), the W stream owns the rest of the sync
            # head, and x kt1+ rides the scalar queue fire-and-forget.
            # The scalar x pieces are size-ascending, so the SDMA's
            # per-transfer round-robin finishes them in consumption
            # order; all have slack vs the cold-start matmul rate.
            paced_dma(actA[:, 0:1, :], xTv[:, 0:1, :])
            for ks in (slice(1, 2), slice(2, 3), slice(3, 4),
                       slice(4, 6), slice(6, 8)):
                nc.scalar.dma_start(actA[:, ks, :], xTv[:, ks, :])

            def dma_w_tile(wt, src, first=False):
                wflat = wt.rearrange("p kt j -> p (kt j)")

                def w_piece(a, b):
                    paced_dma(wflat[:, a:b], src[:, a:b])

                if first:
                    # fine-grained head so the first matmuls start as
                    # early as the issue cadence allows
                    for a, b in ((0, 512), (512, 1024), (1024, 2048),
                                 (2048, 3072), (3072, 4096),
                                 (4096, 6144), (6144, 8192)):
                        w_piece(a, b)
                    # x kt8-15 joins the paced chain here instead of
                    # the scalar queue: these pieces have the most
                    # slack, and keeping them out of the 10-16us
                    # round-robin window stops them stealing packet
                    # share from the kt4-11 weight pieces.
                    paced_dma(actA[:, 8:12, :], xTv[:, 8:12, :])
                    paced_dma(actA[:, 12:16, :], xTv[:, 12:16, :])
                else:
                    for q in range(NQ):
                        w_piece(q * QW, (q + 1) * QW)

            for l in range(N_LEVELS):
                src = actA if l % 2 == 0 else actB
                dst = actB if l % 2 == 0 else actA
                last = l == N_LEVELS - 1
                for jg in range(JG):
                    if not last:
                        wt = wpool.tile([128, KT, 4 * 128], BF16, tag="w")
                        dma_w_tile(wt, Ws[l][jg], first=(l == 0 and jg == 0))
                        wts = (wt, wt)
                    else:
                        wtA = wpool.tile([128, KT, 4 * 128], BF16, tag="w")
                        dma_w_tile(wtA, W3A[jg])
                        wtB = wpool.tile([128, KT, 4 * 128], BF16, tag="w")
                        dma_w_tile(wtB, W3B[jg])
                        wts = (wtA, wtB)
                    accs = [ps.tile([128, (c0, c1)[c]], F32, tag="ps",
                                    name="acc")
                            for c in range(2) for _ in range(4)]
                    for q in range(NQ):
                        for jj in range(4):
                            for kt in range(q * KQ, (q + 1) * KQ):
                                for c in range(2):
                                    nc.tensor.matmul(
                                        accs[c * 4 + jj][:],
                                        wts[c][:, kt,
                                               jj * 128:(jj + 1) * 128],
                                        src[:, kt, csl[c]],
                                        start=(kt == 0),
                                        stop=(kt == KT - 1),
                                    )
                    if not last:
                        for c in range(2):
                            for jj in range(4):
                                jt = jg * 4 + jj
                                nc.scalar.activation(
                                    dst[:, jt, csl[c]], accs[c * 4 + jj][:],
                                    mybir.ActivationFunctionType.Relu,
                                    bias=btile[:, l, jt:jt + 1],
                                )
                    elif jg < JG - 1:
                        for c in range(2):
                            for jj in range(4):
                                jt = jg * 4 + jj
                                nc.scalar.activation(
                                    dst[:, jt, csl[c]], accs[c * 4 + jj][:],
                                    mybir.ActivationFunctionType.Relu,
                                    bias=btile[:, 3 + c, jt:jt + 1],
                                )
                        # ship this jg's four feature blocks on the
                        # scalar HW queue: the scalar engine itself
                        # produced the ACTs this store waits on, so the
                        # issue never head-of-line blocks anything (a
                        # sync-chain store would stall the W3 stream;
                        # SWDGE is slow and delays the teardown).
                        nc.scalar.dma_start(
                            outv[:, jg * 4:(jg + 1) * 4, :],
                            dst[:, jg * 4:(jg + 1) * 4, :])
                    else:
                        # last jg: activation per (jt, chunk) pair and
                        # store per jt row as soon as both chunks are
                        # done, so the tail is one small transfer.
                        for jj in range(4):
                            jt = jg * 4 + jj
                            for c in range(2):
                                nc.scalar.activation(
                                    dst[:, jt, csl[c]], accs[c * 4 + jj][:],
                                    mybir.ActivationFunctionType.Relu,
                                    bias=btile[:, 3 + c, jt:jt + 1],
                                )
                            nc.scalar.dma_start(outv[:, jt, :],
                                                dst[:, jt, :])

    nc.compile()
    _cache[key] = nc
    return nc


def _linearize_w(W: np.ndarray) -> np.ndarray:
    """[D, D] -> bf16 [JG, 128, KT*512] with
    (jg, p, kt, jc) = W[kt*128+p, jg*512+jc]."""
    return np.ascontiguousarray(
        W.astype(BF16_NP).reshape(KT, 128, JG, 512)
        .transpose(2, 1, 0, 3).reshape(JG, 128, KT * 512))


def _linearize_bias(bs: list[np.ndarray]) -> np.ndarray:
    """5 x [D] fp32 -> [128, 5*JT] with (p, l*JT+jt) = bs[l][jt*128+p]."""
    arr = np.stack(bs).astype(np.float32)          # [5, D]
    return np.ascontiguousarray(
        arr.reshape(N_LEVELS + 1, JT, 128).transpose(2, 0, 1)
        .reshape(128, (N_LEVELS + 1) * JT))


def kernel(x, path_mask, W0, b0, W1, b1, W2, b2, W3, b3, _trace=False):
    x = np.asarray(x, dtype=np.float32)
    Wls = [np.asarray(W, dtype=np.float32) for W in (W0, W1, W2, W3)]
    bls = [np.asarray(b, dtype=np.float32) for b in (b0, b1, b2, b3)]
    B = x.shape[0]

    pm = np.asarray(path_mask)
    e3 = (pm[:, 0] * 4 + pm[:, 1] * 2 + pm[:, 2]).astype(np.int64)
    leaf_rows = [np.nonzero(e3 == e)[0] for e in range(8)]
    counts = np.array([len(r) for r in leaf_rows])

    # per-core chunk assignment: core 2j+k gets half k of leaf A_j in
    # chunk 0 and half k of leaf B_j in chunk 1, where (A_j, B_j) is
    # (2j, 2j+1) possibly swapped -- the swap mask is chosen to
    # minimize the compiled c0+c1 (per-core column count).
    halves = [None] * 8  # halves[leaf] = (rows_half0, rows_half1)
    for e in range(8):
        hh = (counts[e] + 1) // 2
        halves[e] = (leaf_rows[e][:hh], leaf_rows[e][hh:])
    hmax = [(counts[e] + 1) // 2 for e in range(8)]

    def even(v):
        return max(2, (int(v) + 1) // 2 * 2)

    best = None
    for mask in range(16):
        A = [2 * j + ((mask >> j) & 1) for j in range(4)]
        Bb = [2 * j + 1 - ((mask >> j) & 1) for j in range(4)]
        a = even(max(hmax[e] for e in A))
        b = even(max(hmax[e] for e in Bb))
        if best is None or a + b < best[0]:
            best = (a + b, mask, a, b)
    _, mask, c0, c1 = best
    Aleaf = [2 * j + ((mask >> j) & 1) for j in range(4)]
    Bleaf = [2 * j + 1 - ((mask >> j) & 1) for j in range(4)]
    # nseg > 1 only under extreme routing skew (a leaf with > 1024
    # rows); each extra segment re-runs the kernel on the overflow.
    nseg = max(1, math.ceil(c0 / 512), math.ceil(c1 / 512))
    c0 = min(c0, 512)
    c1 = min(c1, 512)
    Bc = c0 + c1
    nc = _build(c0, c1)

    xT_bf = np.ascontiguousarray(x.T.astype(BF16_NP))
    in_common = []
    for cid in range(N_CORES):
        j = cid // 2
        eids = (0, j >> 1, j)
        m = {f"W{l}": _linearize_w(Wls[l][eids[l]]) for l in range(3)}
        m["W3A"] = _linearize_w(Wls[3][Aleaf[j]])
        m["W3B"] = _linearize_w(Wls[3][Bleaf[j]])
        m["bias"] = _linearize_bias(
            [bls[0][0], bls[1][j >> 1], bls[2][j],
             bls[3][Aleaf[j]], bls[3][Bleaf[j]]])
        in_common.append(m)

    core_groups = []
    for cid in range(N_CORES):
        j, k = cid // 2, cid % 2
        core_groups.append((halves[Aleaf[j]][k], halves[Bleaf[j]][k]))

    out_full = np.zeros((B, D), dtype=np.float32)
    last_res = None
    for s in range(nseg):
        in_maps = []
        segs = []
        for cid in range(N_CORES):
            g0, g1 = core_groups[cid]
            g0 = g0[s * c0:(s + 1) * c0]
            g1 = g1[s * c1:(s + 1) * c1]
            segs.append((g0, g1))
            xTc = np.zeros((D, Bc), dtype=BF16_NP)
            xTc[:, :len(g0)] = xT_bf[:, g0]
            xTc[:, c0:c0 + len(g1)] = xT_bf[:, g1]
            # [D, Bc] -> SBUF-layout [128, KT*Bc]
            xlin = np.ascontiguousarray(
                xTc.reshape(KT, 128, Bc).transpose(1, 0, 2)
                .reshape(128, KT * Bc))
            in_maps.append({"xT": xlin, **in_common[cid]})
        res = run_bass_kernel_spmd(nc, in_maps, list(range(N_CORES)),
                                   trace=_trace)
        last_res = res
        for cid in range(N_CORES):
            g0, g1 = segs[cid]
            # SBUF-layout [128, JT*Bc] -> [D, Bc]
            o = res.results[cid]["out"].reshape(128, JT, Bc)
            o = o.transpose(1, 0, 2).reshape(D, Bc)
            out_full[g0] = o[:, :len(g0)].astype(np.float32).T
            out_full[g1] = o[:, c0:c0 + len(g1)].astype(np.float32).T
    if _trace:
        return out_full, last_res
    return out_full
